# revision 3
# baseline (speedup 1.0000x reference)
"""MoE experts kernel for Trainium2 (8 NeuronCores, expert-parallel).

Problem (nn_MoEExperts): T=2048 tokens, H=768 hidden, E=8 experts,
F=2048 ffn dim, top-2 routing.

    out[t] = sum_e cw[t,e] * ( gelu(x[t] @ w1[e].T) * (x[t] @ v1[e].T) ) @ w2[e]

Sharding: expert-parallel - core e holds expert e's three weight matrices.
Token dispatch by top_experts happens host-side: tokens routed to expert e
are gathered (pre-transposed) into that core's input, padded to a common
capacity C so all 8 cores run one SPMD program.  The combine (scale by
routing weight + scatter-add over experts) happens host-side on the 8
returned per-expert outputs.

Precision: phase-1 matmuls run in fp8e4 (e4m3) with DoubleRow perf mode
(two k-tiles contracted per instruction at 0.5 cycles/row).  To hold
accuracy, x / w1 / v1 use a TWO-TERM fp8 representation: value = hi + res
where both terms are quantized at the SAME scale (e4m3's wide dynamic
range keeps the ~30x smaller residual in normal range).  The product
expands to three matmul-sets per PSUM group, dropping the res*res term:

    h = (Wh + Wr)^T xh + Wh^T xr       (9 DoubleRow matmuls per tile)

All terms share one scale (Sx*Sw) so they accumulate natively in PSUM.
Sx*Sw = 4096 is chosen small enough that glu_raw = gelu(h1_true) * h2_raw
fits fp16, so the only dequant is a single scalar folded into the
host-side combine.  Phase 2 (out = glu @ W2) runs in fp16.

Device program per core:
  phase 1:  h1T = W1 @ xT, h2T = V1 @ xT   ([F, C] tiles, K=H, two-term fp8)
            g1 = gelu(h1T / (Sx*Sw))       (ACT exact-erf Gelu, scale folded)
            gluT = g1 * h2T                (DVE mul -> fp16, scale Sx*Sw)
  phase 2:  outT = W2.T @ gluT             ([H, C], K=F, fp16)
"""

import os
import sys

if "/opt/trn_rl_repo" not in sys.path:
    sys.path.insert(0, "/opt/trn_rl_repo")

import numpy as np
import ml_dtypes

E = 8
F = 2048
H = 768
TOPK = 2
P = 128
FT = F // P   # 16
KT = H // P   # 6
KP = KT // 2  # 3 k-pairs for DoubleRow
HT = H // P   # 6
WV_SLABS = [1, 1, 2, 2, 2, 2, 2, 2, 2]  # f-tiles per w1/v1 DMA slab (sum=16)

SX = 32.0     # x fp8 scale
SW = 128.0    # w1/v1 fp8 scale
DEQ = 1.0 / (SX * SW)   # folded into host combine

# phase-1 matmul sets: (weight term, x term); hi=0, res=1.  (res,res) dropped.
PH1_SETS = [(0, 0), (0, 1), (1, 0)]

# Set by kernel() when KERNEL_TRACE=1.
LAST_EXEC_NS = None
LAST_MEAN_EXEC_NS = None
LAST_RESULTS = None

F8 = ml_dtypes.float8_e4m3   # 240 max; bit-compatible with e4m3fn below 240


def _chunks(c):
    """Split c columns into moving-dim chunks <=512 (and >=256 when
    possible, so matmuls keep full rate)."""
    out = []
    rem = c
    while rem > 512:
        take = rem - 256 if (rem - 512 < 256 and rem < 1024) else 512
        out.append(take)
        rem -= take
    out.append(rem)
    return out


def _q8_two(a, scale):
    """Two-term fp8 quantization at a common scale: hi + res ~ a*scale."""
    s = (np.asarray(a, np.float32) * scale).astype(np.float32)
    hi = np.clip(s, -240.0, 240.0).astype(F8)
    res = np.clip(s - hi.astype(np.float32), -240.0, 240.0).astype(F8)
    return hi, res


def _install_trace_shim():
    """Register the axon NTFF profile hook (antenv.axon_hooks is missing in
    this image) and neuter the remote artifact upload."""
    import types

    try:
        import antenv.axon_hooks  # noqa: F401
    except ImportError:
        mod = types.ModuleType("antenv.axon_hooks")
        mod._hook = None
        mod.set_axon_ntff_profile_hook = lambda h: setattr(mod, "_hook", h)
        mod.get_axon_ntff_profile_hook = lambda: mod._hook
        sys.modules["antenv.axon_hooks"] = mod
        import antenv

        antenv.axon_hooks = mod
        from trn_agent_boot.trn_boot import _ntff_profile_via_ctypes

        hook = _ntff_profile_via_ctypes("/opt/axon/libaxon_pjrt.so")
        if hook is not None:
            mod.set_axon_ntff_profile_hook(hook)

    import concourse.bass_utils as bu

    bu.upload_artifacts = lambda tmpdir: "local://skipped"


def _build_program(C):
    """SPMD Bass program for per-expert capacity C (multiple of 128)."""
    import concourse.mybir as mybir
    import concourse.tile as tile
    from concourse import bacc

    f32 = mybir.dt.float32
    f16 = mybir.dt.float16
    f8 = mybir.dt.float8e4
    cch = _chunks(C)
    DR = mybir.MatmulPerfMode.DoubleRow

    nc = bacc.Bacc(None, target_bir_lowering=False, debug=False)

    # Host-prepared layouts (partition index first, rows contiguous):
    #   xt [128p, 2s, KT, C]             xt[p,s,k,c]     = xq_s[ids[c], k*128+p]
    #   wv [128p, FT, 2j, 2s, KT, 128f]  wv[p,f,j,s,k,q] = Wj_s[f*128+q, k*128+p]
    #   w2 [128p, FT, H]                 w2[p,t,h]       = W2[t*128+p, h]
    xt_d = nc.declare_dram_parameter("xt", [P, 2, KT, C], f8, isOutput=False)
    wv_d = nc.declare_dram_parameter("wv", [P, FT, 2, 2, KT, P], f8, isOutput=False)
    w2_d = nc.declare_dram_parameter("w2", [P, FT, H], f16, isOutput=False)
    out_d = nc.declare_dram_parameter("out", [H, C], f32, isOutput=True)

    with tile.TileContext(nc) as tc:
        with tc.tile_pool(name="persist", bufs=1) as persist, \
             tc.tile_pool(name="osb", bufs=4) as osb_pool, \
             tc.tile_pool(name="gtmp", bufs=3) as gtmp, \
             tc.tile_pool(name="ps1", bufs=2, space="PSUM") as ps1, \
             tc.tile_pool(name="ps2", bufs=4, space="PSUM") as ps2:

            # xt on the scalar HWDGE queue so its transfer runs in parallel
            # with the first weight slab pushed on the sync queue; hi term
            # first so the first matmuls can start sooner.
            xt_sb = persist.tile([P, 2, KT, C], f8, tag="xt", name="xt_sb")
            nc.sync.dma_start(out=xt_sb[:, 0], in_=xt_d.ap()[:, 0])
            nc.scalar.dma_start(out=xt_sb[:, 1], in_=xt_d.ap()[:, 1])

            # Pre-warm the PE (HAM clock gate) with throwaway matmuls while
            # the first input DMAs are in flight: by the time real data
            # lands, the PE runs at 2.4 GHz instead of 1.2.
            dummy = gtmp.tile([P, 512], f16, tag="dummy", name="dummy")
            nc.vector.memset(dummy, 0.0)
            for wi in range(10):
                d_ps = ps2.tile([P, 512], f32, tag="ops", name=f"warm{wi}")
                nc.tensor.matmul(d_ps[:], dummy[:, :P], dummy[:],
                                 start=True, stop=True)

            wv_sb = []   # per f-tile: (tile, index within slab)
            f0 = 0
            for b, nf in enumerate(WV_SLABS):
                t = persist.tile([P, nf, 2, 2, KT, P], f8,
                                 tag=f"wv{b}", name=f"wv{b}")
                if b == 0:
                    # w1-hi[f0] gates the very first matmul group -- land it
                    # before the res term and v1
                    nc.sync.dma_start(out=t[:, :, 0, 0], in_=wv_d.ap()[:, 0:nf, 0, 0])
                    nc.sync.dma_start(out=t[:, :, 0, 1], in_=wv_d.ap()[:, 0:nf, 0, 1])
                    nc.sync.dma_start(out=t[:, :, 1], in_=wv_d.ap()[:, 0:nf, 1])
                else:
                    nc.sync.dma_start(out=t, in_=wv_d.ap()[:, f0:f0 + nf])
                for fi in range(nf):
                    wv_sb.append((t, fi))
                f0 += nf

            # w2 is only needed in phase 2 -- queue it after the phase-1 weights
            w2_sb = persist.tile([P, FT, H], f16, tag="w2", name="w2_sb")
            nc.sync.dma_start(out=w2_sb, in_=w2_d.ap())

            glu_sb = persist.tile([P, FT, C], f16, tag="glu", name="glu_sb")

            # ---- phase 1: gluT[F, C] = gelu(W1 @ xT / s) * (V1 @ xT) ----
            # Each h1/h2 accumulation: 3 two-term sets x KP DoubleRow matmuls.
            for f in range(FT):
                blk, fi = wv_sb[f]
                col = 0
                for ch in cch:
                    h1 = ps1.tile([P, ch], f32, tag="h1", name=f"h1_{f}_{col}")
                    h2 = ps1.tile([P, ch], f32, tag="h2", name=f"h2_{f}_{col}")
                    for j, hp in ((0, h1), (1, h2)):
                        n = 0
                        for (ws, xs) in PH1_SETS:
                            for k in range(KP):
                                n += 1
                                nc.tensor.matmul(
                                    hp[:],
                                    blk[:, fi, j, ws, 2 * k:2 * k + 2, :],
                                    xt_sb[:, xs, 2 * k:2 * k + 2, col:col + ch],
                                    start=(n == 1),
                                    stop=(n == len(PH1_SETS) * KP),
                                    perf_mode=DR)
                    g1 = gtmp.tile([P, ch], f32, tag="g1", name=f"g1_{f}_{col}")
                    nc.scalar.activation(g1[:], h1[:],
                                         mybir.ActivationFunctionType.Gelu,
                                         scale=DEQ)
                    nc.vector.tensor_mul(glu_sb[:, f, col:col + ch], g1[:], h2[:])
                    col += ch

            # ---- phase 2: outT[H, C] = W2.T @ gluT ----
            for h in range(HT):
                col = 0
                cch_h = cch
                if h >= HT - 2:
                    cch_h = []
                    for ch in cch:
                        if ch > 256:
                            cch_h += [ch - ch // 2, ch // 2]
                        else:
                            cch_h.append(ch)
                for ch in cch_h:
                    o_ps = ps2.tile([P, ch], f32, tag="ops", name=f"o_{h}_{col}")
                    for k in range(FT):
                        nc.tensor.matmul(o_ps[:],
                                         w2_sb[:, k, h * P:(h + 1) * P],
                                         glu_sb[:, k, col:col + ch],
                                         start=(k == 0), stop=(k == FT - 1))
                    o_sb = osb_pool.tile([P, ch], f32, tag="osb",
                                         name=f"os_{h}_{col}")
                    eng = nc.sync if (h + col // 256) % 2 == 0 else nc.scalar
                    nc.vector.tensor_copy(o_sb[:], o_ps[:])
                    eng.dma_start(
                        out=out_d.ap()[h * P:(h + 1) * P, col:col + ch],
                        in_=o_sb[:])
                    col += ch

    nc.compile()
    return nc


def kernel(x, top_weights, w1, v1, w2, top_experts):
    global LAST_EXEC_NS, LAST_MEAN_EXEC_NS, LAST_RESULTS

    from concourse.bass_utils import run_bass_kernel_spmd

    x = np.asarray(x)
    bsz, q_len, hidden = x.shape
    T = bsz * q_len
    x2 = np.ascontiguousarray(x.reshape(T, hidden).astype(np.float32, copy=False))
    te = np.asarray(top_experts).astype(np.int64, copy=False)
    tw = np.asarray(top_weights).astype(np.float32, copy=False)
    w1r = np.asarray(w1, dtype=np.float32).reshape(E, F, H)
    v1r = np.asarray(v1, dtype=np.float32).reshape(E, F, H)
    w2r = np.asarray(w2, dtype=np.float32).reshape(E, F, H)

    # Host-side dispatch: combine weights per (token, expert) summed over
    # top-k slots (handles duplicate experts within a token's top-k).
    cw = np.zeros((T, E), np.float32)
    rows = np.repeat(np.arange(T), TOPK)
    np.add.at(cw, (rows, te.reshape(-1)), tw.reshape(-1))

    ids = [np.nonzero((te == e).any(axis=1))[0] for e in range(E)]
    counts = [len(i) for i in ids]
    C = max(256, -(-max(counts) // P) * P)

    in_maps = []
    for e in range(E):
        xg = np.zeros((C, H), np.float32)
        ce = counts[e]
        if ce:
            xg[:ce] = x2[ids[e]]
        xhi, xres = _q8_two(xg, SX)
        # xt[p, s, k, c] = xq_s[c, k*128+p]
        xt = np.ascontiguousarray(np.stack(
            [a.reshape(C, KT, P).transpose(2, 0, 1) for a in (xhi, xres)],
            axis=1).transpose(0, 1, 3, 2))
        # wv[p, f, j, s, k, q] = Wj_s[f*128+q, k*128+p]
        terms = []
        for wmat in (w1r[e], v1r[e]):
            hi, res = _q8_two(wmat, SW)
            terms.append(np.stack(
                [a.reshape(FT, P, KT, P).transpose(3, 0, 2, 1) for a in (hi, res)],
                axis=0))
        wv = np.ascontiguousarray(
            np.stack(terms, axis=0).transpose(2, 3, 0, 1, 4, 5))
        # w2h[p, t, h] = W2[t*128+p, h]
        w2h = np.ascontiguousarray(
            w2r[e].astype(np.float16).reshape(FT, P, H).transpose(1, 0, 2))
        in_maps.append({"xt": xt, "wv": wv, "w2": w2h})

    nc = _build_program(C)

    trace = os.environ.get("KERNEL_TRACE", "") == "1"
    if trace:
        _install_trace_shim()
        res = run_bass_kernel_spmd(nc, in_maps, list(range(E)),
                                   trace=True, trace_cores=list(range(E)))
        LAST_EXEC_NS = res.exec_time_ns
        LAST_MEAN_EXEC_NS = res.mean_exec_time_ns
        LAST_RESULTS = res
    else:
        res = run_bass_kernel_spmd(nc, in_maps, list(range(E)))

    # Host-side combine: scale each expert's rows by its routing weight and
    # scatter-add back to token order (fp8 dequant scale folded in).
    out = np.zeros((T, H), np.float32)
    for e in range(E):
        ce = counts[e]
        if not ce:
            continue
        oe = res.results[e]["out"][:, :ce].T  # [ce, H]
        out[ids[e]] += oe * (cw[ids[e], e] * DEQ)[:, None]

    return out.reshape(bsz, q_len, hidden).astype(np.float32, copy=False)


# revision 6
# speedup vs baseline: 1.2009x; 1.2009x over previous
"""MoE experts kernel for Trainium2 (8 NeuronCores, expert-parallel).

Problem (nn_MoEExperts): T=2048 tokens, H=768 hidden, E=8 experts,
F=2048 ffn dim, top-2 routing.

    out[t] = sum_e cw[t,e] * ( gelu(x[t] @ w1[e].T) * (x[t] @ v1[e].T) ) @ w2[e]

Sharding: expert-parallel - core e holds expert e's three weight matrices
(each streamed from HBM exactly once).  Token dispatch by top_experts
happens host-side: tokens routed to expert e are gathered (pre-transposed)
into that core's input, padded to a common capacity C so all 8 cores run
one SPMD program.  The combine (scale by routing weight + scatter-add over
experts) happens host-side on the 8 returned per-expert outputs.

Matmul operands are fp16 (fp32 PSUM accumulation; ~5e-4 relative error,
full-rate 1 cycle/row on the tensor engine).  fp8 DoubleRow was measured
at only 2x fp16 per contraction on TRN2 hardware, which makes any
accuracy-preserving two-term fp8 scheme 1.5x SLOWER than fp16 - so fp16
everywhere is the optimal precision here (PE-bound kernel).

Device program per core:
  phase 1:  h1T = W1 @ xT, h2T = V1 @ xT   ([F, C] tiles, K=H, PSUM accum)
            gluT = gelu(h1T) * h2T         (ACT exact-erf Gelu + DVE mul)
  phase 2:  outT = H2 @ gluT = W2.T @ gluT ([H, C], K=F)

Startup is latency-tuned: the very first matmul group only needs w1[f0]
(sync queue, alone) and xt (split across the gpsimd and vector HWDGE
queues, which are idle at t=0; the scalar engine is busy with Gelu
ACT_TABLE_LOADs early).  Warmup matmuls keep the PE busy from t~0.3us so
the HAM clock ramp completes before real work.  The NEFF's fixed
zero-all-semaphores epilogue (~250 serialized ops, ~8us) is shrunk by
capping the compiler's semaphore space with --max-sem-num.
"""

import os
import sys

if "/opt/trn_rl_repo" not in sys.path:
    sys.path.insert(0, "/opt/trn_rl_repo")

import numpy as np

E = 8
F = 2048
H = 768
TOPK = 2
P = 128
FT = F // P   # 16
KT = H // P   # 6
HT = H // P   # 6
# f-tiles per w1/v1 DMA slab after the four initial single-matrix slabs
WV_SLABS = [2, 2, 2, 2, 2, 2, 2]  # sum = 14 (f2..f15)
N_WARMUP = 40
MAX_SEM_NUM = 64

# Set by kernel() when KERNEL_TRACE=1.
LAST_EXEC_NS = None
LAST_MEAN_EXEC_NS = None
LAST_RESULTS = None


def _chunks(c):
    """Split c columns into moving-dim chunks <=512 (and >=256 when
    possible, so matmuls keep full rate)."""
    out = []
    rem = c
    while rem > 512:
        take = rem - 256 if (rem - 512 < 256 and rem < 1024) else 512
        out.append(take)
        rem -= take
    out.append(rem)
    return out


def _install_trace_shim():
    """Register the axon NTFF profile hook (antenv.axon_hooks is missing in
    this image) and neuter the remote artifact upload."""
    import types

    try:
        import antenv.axon_hooks  # noqa: F401
    except ImportError:
        mod = types.ModuleType("antenv.axon_hooks")
        mod._hook = None
        mod.set_axon_ntff_profile_hook = lambda h: setattr(mod, "_hook", h)
        mod.get_axon_ntff_profile_hook = lambda: mod._hook
        sys.modules["antenv.axon_hooks"] = mod
        import antenv

        antenv.axon_hooks = mod
        from trn_agent_boot.trn_boot import _ntff_profile_via_ctypes

        hook = _ntff_profile_via_ctypes("/opt/axon/libaxon_pjrt.so")
        if hook is not None:
            mod.set_axon_ntff_profile_hook(hook)

    import concourse.bass_utils as bu

    bu.upload_artifacts = lambda tmpdir: "local://skipped"


def _build_program(C):
    """SPMD Bass program for per-expert capacity C (multiple of 128)."""
    import concourse.mybir as mybir
    import concourse.tile as tile
    from concourse import bacc

    f32 = mybir.dt.float32
    mdt = mybir.dt.float16
    cch = _chunks(C)

    nc = bacc.Bacc(None, target_bir_lowering=False, debug=False)

    # Host-prepared layouts (partition index first, rows contiguous):
    #   xt [128p, KT, C]            xt[p,k,c]    = x[ids[c], k*128+p]
    #   wv [128p, FT, 2, KT, 128f]  wv[p,f,j,k,q]= Wj[f*128+q, k*128+p]
    #   w2 [128p, FT, H]            w2[p,s,h]    = W2[s*128+p, h]
    xt_d = nc.declare_dram_parameter("xt", [P, KT, C], mdt, isOutput=False)
    wv_d = nc.declare_dram_parameter("wv", [P, FT, 2, KT, P], mdt, isOutput=False)
    w2_d = nc.declare_dram_parameter("w2", [P, FT, H], mdt, isOutput=False)
    out_d = nc.declare_dram_parameter("out", [H, C], f32, isOutput=True)

    with tile.TileContext(nc) as tc:
        with tc.tile_pool(name="persist", bufs=1) as persist, \
             tc.tile_pool(name="osb", bufs=4) as osb_pool, \
             tc.tile_pool(name="gtmp", bufs=3) as gtmp, \
             tc.tile_pool(name="ps1", bufs=2, space="PSUM") as ps1, \
             tc.tile_pool(name="ps2", bufs=4, space="PSUM") as ps2:

            # xt on the gpsimd + scalar HWDGE queues so the transfers run
            # in parallel with w1[f0] on the sync queue.  gpsimd gets the
            # bigger share: the scalar engine is busy early with Gelu
            # ACT_TABLE_LOADs.
            xt_sb = persist.tile([P, KT, C], mdt, tag="xt", name="xt_sb")
            nc.gpsimd.dma_start(out=xt_sb[:, 0:4], in_=xt_d.ap()[:, 0:4])
            nc.scalar.dma_start(out=xt_sb[:, 4:6], in_=xt_d.ap()[:, 4:6])

            # Pre-warm the PE (HAM clock gate) with small throwaway matmuls
            # while the first input DMAs are in flight: by the time real
            # data lands, the PE clock ramp is already under way.
            dummy = gtmp.tile([P, P], mdt, tag="dummy", name="dummy")
            nc.gpsimd.memset(dummy, 0.0)
            for wi in range(N_WARMUP):
                d_ps = ps2.tile([P, 512], f32, tag="ops", name=f"warm{wi}")
                nc.tensor.matmul(d_ps[:, :P], dummy[:], dummy[:],
                                 start=True, stop=True)

            # Weights on the sync queue.  w1[f0] alone gates the very first
            # matmul group -- land it first, then v1[f0], then f1, then the
            # paired slabs.
            wv_sb = []   # per f-tile: (tile, index within slab)
            head = []
            for f in (0, 1):
                t = persist.tile([P, 1, 2, KT, P], mdt,
                                 tag=f"wvh{f}", name=f"wvh{f}")
                head.append(t)
                wv_sb.append((t, 0))
            nc.sync.dma_start(out=head[0][:, :, 0], in_=wv_d.ap()[:, 0:1, 0])
            nc.sync.dma_start(out=head[0][:, :, 1], in_=wv_d.ap()[:, 0:1, 1])
            nc.sync.dma_start(out=head[1][:, :, 0], in_=wv_d.ap()[:, 1:2, 0])
            nc.sync.dma_start(out=head[1][:, :, 1], in_=wv_d.ap()[:, 1:2, 1])
            f0 = 2
            for b, nf in enumerate(WV_SLABS):
                t = persist.tile([P, nf, 2, KT, P], mdt,
                                 tag=f"wv{b}", name=f"wv{b}")
                nc.sync.dma_start(out=t, in_=wv_d.ap()[:, f0:f0 + nf])
                for fi in range(nf):
                    wv_sb.append((t, fi))
                f0 += nf

            # w2 is only needed in phase 2 -- queue it after the phase-1 weights
            w2_sb = persist.tile([P, FT, H], mdt, tag="w2", name="w2_sb")
            nc.sync.dma_start(out=w2_sb, in_=w2_d.ap())

            glu_sb = persist.tile([P, FT, C], mdt, tag="glu", name="glu_sb")

            # ---- phase 1: gluT[F, C] = gelu(W1 @ xT) * (V1 @ xT) ----
            for f in range(FT):
                blk, fi = wv_sb[f]
                col = 0
                for ch in cch:
                    h1 = ps1.tile([P, ch], f32, tag="h1", name=f"h1_{f}_{col}")
                    h2 = ps1.tile([P, ch], f32, tag="h2", name=f"h2_{f}_{col}")
                    for k in range(KT):
                        nc.tensor.matmul(h1[:], blk[:, fi, 0, k, :],
                                         xt_sb[:, k, col:col + ch],
                                         start=(k == 0), stop=(k == KT - 1))
                    for k in range(KT):
                        nc.tensor.matmul(h2[:], blk[:, fi, 1, k, :],
                                         xt_sb[:, k, col:col + ch],
                                         start=(k == 0), stop=(k == KT - 1))
                    g1 = gtmp.tile([P, ch], f32, tag="g1", name=f"g1_{f}_{col}")
                    nc.scalar.activation(g1[:], h1[:],
                                         mybir.ActivationFunctionType.Gelu)
                    nc.vector.tensor_mul(glu_sb[:, f, col:col + ch], g1[:], h2[:])
                    col += ch

            # ---- phase 2: outT[H, C] = W2.T @ gluT ----
            for h in range(HT):
                col = 0
                cch_h = cch
                if h >= HT - 2:
                    cch_h = []
                    for ch in cch:
                        if ch > 256:
                            cch_h += [ch - ch // 2, ch // 2]
                        else:
                            cch_h.append(ch)
                for ch in cch_h:
                    o_ps = ps2.tile([P, ch], f32, tag="ops", name=f"o_{h}_{col}")
                    for k in range(FT):
                        nc.tensor.matmul(o_ps[:],
                                         w2_sb[:, k, h * P:(h + 1) * P],
                                         glu_sb[:, k, col:col + ch],
                                         start=(k == 0), stop=(k == FT - 1))
                    o_sb = osb_pool.tile([P, ch], f32, tag="osb",
                                         name=f"os_{h}_{col}")
                    eng = nc.sync if (h + col // 256) % 2 == 0 else nc.scalar
                    nc.vector.tensor_copy(o_sb[:], o_ps[:])
                    eng.dma_start(
                        out=out_d.ap()[h * P:(h + 1) * P, col:col + ch],
                        in_=o_sb[:])
                    col += ch

    nc.compile()
    return nc


def kernel(x, top_weights, w1, v1, w2, top_experts):
    global LAST_EXEC_NS, LAST_MEAN_EXEC_NS, LAST_RESULTS

    import concourse.bass_utils as bu
    from concourse.bass_utils import run_bass_kernel_spmd

    npdt = np.float16

    x = np.asarray(x)
    bsz, q_len, hidden = x.shape
    T = bsz * q_len
    x2 = np.ascontiguousarray(x.reshape(T, hidden).astype(np.float32, copy=False))
    te = np.asarray(top_experts).astype(np.int64, copy=False)
    tw = np.asarray(top_weights).astype(np.float32, copy=False)
    w1r = np.asarray(w1, dtype=np.float32).reshape(E, F, H)
    v1r = np.asarray(v1, dtype=np.float32).reshape(E, F, H)
    w2r = np.asarray(w2, dtype=np.float32).reshape(E, F, H)

    # Host-side dispatch: combine weights per (token, expert) summed over
    # top-k slots (handles duplicate experts within a token's top-k).
    cw = np.zeros((T, E), np.float32)
    rows = np.repeat(np.arange(T), TOPK)
    np.add.at(cw, (rows, te.reshape(-1)), tw.reshape(-1))

    ids = [np.nonzero((te == e).any(axis=1))[0] for e in range(E)]
    counts = [len(i) for i in ids]
    C = max(256, -(-max(counts) // P) * P)

    in_maps = []
    for e in range(E):
        xg = np.zeros((C, H), npdt)
        ce = counts[e]
        if ce:
            xg[:ce] = x2[ids[e]].astype(npdt)
        # xt[p, k, c] = xg[c, k*128+p]
        xt = np.ascontiguousarray(xg.reshape(C, KT, P).transpose(2, 1, 0))
        # wv[p, f, j, k, q] = Wj[e][f*128+q, k*128+p]
        w1t = w1r[e].astype(npdt).reshape(FT, P, KT, P).transpose(3, 0, 2, 1)
        v1t = v1r[e].astype(npdt).reshape(FT, P, KT, P).transpose(3, 0, 2, 1)
        wv = np.ascontiguousarray(np.stack([w1t, v1t], axis=2))
        # w2h[p, s, h] = W2[e][s*128+p, h]
        w2h = np.ascontiguousarray(
            w2r[e].astype(npdt).reshape(FT, P, H).transpose(1, 0, 2))
        in_maps.append({"xt": xt, "wv": wv, "w2": w2h})

    nc = _build_program(C)

    # Cap the compiler's semaphore space: the NEFF epilogue serially zeroes
    # every allocatable semaphore (~250 ops ~ 8us at the default 256).
    orig_walrus_args = bu.get_walrus_args

    def _walrus_args(*a, **k):
        return orig_walrus_args(*a, **k) + [f"--max-sem-num={MAX_SEM_NUM}"]

    bu.get_walrus_args = _walrus_args
    try:
        trace = os.environ.get("KERNEL_TRACE", "") == "1"
        if trace:
            _install_trace_shim()
            res = run_bass_kernel_spmd(nc, in_maps, list(range(E)),
                                       trace=True, trace_cores=list(range(E)))
            LAST_EXEC_NS = res.exec_time_ns
            LAST_MEAN_EXEC_NS = res.mean_exec_time_ns
            LAST_RESULTS = res
        else:
            res = run_bass_kernel_spmd(nc, in_maps, list(range(E)))
    finally:
        bu.get_walrus_args = orig_walrus_args

    # Host-side combine: scale each expert's rows by its routing weight and
    # scatter-add back to token order.
    out = np.zeros((T, H), np.float32)
    for e in range(E):
        ce = counts[e]
        if not ce:
            continue
        oe = res.results[e]["out"][:, :ce].T  # [ce, H]
        out[ids[e]] += oe * cw[ids[e], e][:, None]

    return out.reshape(bsz, q_len, hidden).astype(np.float32, copy=False)


# revision 15
# speedup vs baseline: 1.2308x; 1.0248x over previous
"""MoE experts kernel for Trainium2 (8 NeuronCores, expert-parallel).

Problem (nn_MoEExperts): T=2048 tokens, H=768 hidden, E=8 experts,
F=2048 ffn dim, top-2 routing.

    out[t] = sum_e cw[t,e] * ( gelu(x[t] @ w1[e].T) * (x[t] @ v1[e].T) ) @ w2[e]

Sharding: expert-parallel - core e holds expert e's three weight matrices
(each streamed from HBM exactly once).  Token dispatch by top_experts
happens host-side: tokens routed to expert e are gathered (pre-transposed)
into that core's input, padded to a common capacity C so all 8 cores run
one SPMD program.  The combine (scale by routing weight + scatter-add over
experts) happens host-side on the 8 returned per-expert outputs.

Matmul operands are fp16 (fp32 PSUM accumulation; ~5e-4 relative error,
full-rate 1 cycle/row on the tensor engine).  fp8 DoubleRow was measured
at only 2x fp16 per contraction on TRN2 hardware, which makes any
accuracy-preserving two-term fp8 scheme 1.5x SLOWER than fp16 - so fp16
everywhere is the optimal precision here (PE-bound kernel).

Device program per core:
  phase 1:  h1T = W1 @ xT, h2T = V1 @ xT   ([F, C] tiles, K=H, PSUM accum)
            gluT = gelu(h1T) * h2T         (ACT exact-erf Gelu + DVE mul)
  phase 2:  outT = H2 @ gluT = W2.T @ gluT ([H, C], K=F)

Startup is latency-tuned: the very first matmul group only needs w1[f0]
(sync queue, alone) and xt (split across the gpsimd and vector HWDGE
queues in consumption order; the scalar queue is delayed ~1.2us by the
Gelu ACT_TABLE_LOADs).  Warmup matmuls keep the PE busy from t~0.3us so
the HAM clock ramp completes before real work.  (The NEFF's fixed
zero-all-semaphores epilogue, ~250 serialized ops / ~9us, is compiler
boilerplate: --max-sem-num was A/B-tested to have no effect on it.)
"""

import os
import sys

if "/opt/trn_rl_repo" not in sys.path:
    sys.path.insert(0, "/opt/trn_rl_repo")

import numpy as np

E = 8
F = 2048
H = 768
TOPK = 2
P = 128
FT = F // P   # 16
KT = H // P   # 6
HT = H // P   # 6
# f-tiles per weight slab: singles early (fine-grained deps for the
# startup transient), wider once the pipeline is ahead.  sum = 16.
WV_SLABS = [1, 1, 1, 1, 4, 4, 4]
N_WARMUP = 20   # 8 x 512-col + 12 x 128-col

# Set by kernel() when KERNEL_TRACE=1.
LAST_EXEC_NS = None
LAST_MEAN_EXEC_NS = None
LAST_RESULTS = None


def _chunks(c):
    """Split c columns into moving-dim chunks <=512 (and >=256 when
    possible, so matmuls keep full rate)."""
    out = []
    rem = c
    while rem > 512:
        take = rem - 256 if (rem - 512 < 256 and rem < 1024) else 512
        out.append(take)
        rem -= take
    out.append(rem)
    return out


def _install_trace_shim():
    """Register the axon NTFF profile hook (antenv.axon_hooks is missing in
    this image) and neuter the remote artifact upload."""
    import types

    try:
        import antenv.axon_hooks  # noqa: F401
    except ImportError:
        mod = types.ModuleType("antenv.axon_hooks")
        mod._hook = None
        mod.set_axon_ntff_profile_hook = lambda h: setattr(mod, "_hook", h)
        mod.get_axon_ntff_profile_hook = lambda: mod._hook
        sys.modules["antenv.axon_hooks"] = mod
        import antenv

        antenv.axon_hooks = mod
        from trn_agent_boot.trn_boot import _ntff_profile_via_ctypes

        hook = _ntff_profile_via_ctypes("/opt/axon/libaxon_pjrt.so")
        if hook is not None:
            mod.set_axon_ntff_profile_hook(hook)

    import concourse.bass_utils as bu

    bu.upload_artifacts = lambda tmpdir: "local://skipped"


def _build_program(C):
    """SPMD Bass program for per-expert capacity C (multiple of 128)."""
    import concourse.mybir as mybir
    import concourse.tile as tile
    from concourse import bacc

    f32 = mybir.dt.float32
    mdt = mybir.dt.float16
    cch = _chunks(C)

    nc = bacc.Bacc(None, target_bir_lowering=False, debug=False)

    # Host-prepared layouts (partition index first, rows contiguous):
    #   xt [128p, KT, C]            xt[p,k,c]    = x[ids[c], k*128+p]
    #   wv [128p, FT, 2, KT, 128f]  wv[p,f,j,k,q]= Wj[f*128+q, k*128+p]
    #   w2 [128p, FT, H]            w2[p,s,h]    = W2[s*128+p, h]
    xt_d = nc.declare_dram_parameter("xt", [P, KT, C], mdt, isOutput=False)
    wv_d = nc.declare_dram_parameter("wv", [P, FT, 2, KT, P], mdt, isOutput=False)
    w2_d = nc.declare_dram_parameter("w2", [P, FT, H], mdt, isOutput=False)
    out_d = nc.declare_dram_parameter("out", [H, C], f32, isOutput=True)

    with tile.TileContext(nc) as tc:
        with tc.tile_pool(name="persist", bufs=1) as persist, \
             tc.tile_pool(name="osb", bufs=4) as osb_pool, \
             tc.tile_pool(name="gtmp", bufs=3) as gtmp, \
             tc.tile_pool(name="ps1", bufs=2, space="PSUM") as ps1, \
             tc.tile_pool(name="ps2", bufs=4, space="PSUM") as ps2:

            # Startup DMAs are latency-tuned for the first matmul groups.
            # w1 streams on the sync queue, v1 on the scalar queue (their
            # consumption alternates h1/h2 per f-tile), xt split across
            # both.  The scalar queue starts ~1.2us late (behind the Gelu
            # ACT_TABLE_LOADs), which the order below accounts for.
            xt_sb = persist.tile([P, KT, C], mdt, tag="xt", name="xt_sb")
            slabs = []   # (w1 tile, v1 tile, first f, nf)
            wv_sb = []   # per f-tile: (w1 tile, v1 tile, index in slab)
            f0 = 0
            for b, nf in enumerate(WV_SLABS):
                tw = persist.tile([P, nf, KT, P], mdt, tag=f"w1s{b}",
                                  name=f"w1s{b}")
                tv = persist.tile([P, nf, KT, P], mdt, tag=f"v1s{b}",
                                  name=f"v1s{b}")
                slabs.append((tw, tv, f0, nf))
                for fi in range(nf):
                    wv_sb.append((tw, tv, fi))
                f0 += nf

            # sync: w1[f0] then xt[k0:2] gate the very first group.
            nc.sync.dma_start(out=slabs[0][0], in_=wv_d.ap()[:, 0:1, 0])
            nc.sync.dma_start(out=xt_sb[:, 0:2], in_=xt_d.ap()[:, 0:2])
            # scalar: rest of xt, finest pieces last-needed-first packed.
            nc.scalar.dma_start(out=xt_sb[:, 2:4], in_=xt_d.ap()[:, 2:4])
            nc.scalar.dma_start(out=xt_sb[:, 4:5], in_=xt_d.ap()[:, 4:5])
            nc.scalar.dma_start(out=xt_sb[:, 5:6], in_=xt_d.ap()[:, 5:6])
            # v1[f0] on sync right behind (scalar is still on xt).
            nc.sync.dma_start(out=slabs[0][1], in_=wv_d.ap()[:, 0:1, 1])
            # Remaining slabs: w1 on sync, v1 on scalar, in f order.
            for tw, tv, fs, nf in slabs[1:]:
                nc.sync.dma_start(out=tw, in_=wv_d.ap()[:, fs:fs + nf, 0])
                nc.scalar.dma_start(out=tv, in_=wv_d.ap()[:, fs:fs + nf, 1])

            # w2 is only needed in phase 2 -- queue it after the phase-1 weights
            w2_sb = persist.tile([P, FT, H], mdt, tag="w2", name="w2_sb")
            nc.sync.dma_start(out=w2_sb, in_=w2_d.ap())

            # Pre-warm the PE (HAM clock gate) with throwaway matmuls while
            # the first input DMAs are in flight: by the time real data
            # lands, the PE clock ramp is already under way.  Mostly large
            # tiles for sustained busy, small ones at the end so the queue
            # drains quickly when real work arrives.
            dummy = gtmp.tile([P, 512], mdt, tag="dummy", name="dummy")
            nc.gpsimd.memset(dummy, 0.0)
            for wi in range(N_WARMUP):
                d_ps = ps2.tile([P, 512], f32, tag="ops", name=f"warm{wi}")
                cols = 512 if wi < 8 else P
                nc.tensor.matmul(d_ps[:, :cols], dummy[:, :P], dummy[:, :cols],
                                 start=True, stop=True)

            glu_sb = persist.tile([P, FT, C], mdt, tag="glu", name="glu_sb")

            # ---- phase 1: gluT[F, C] = gelu(W1 @ xT) * (V1 @ xT) ----
            for f in range(FT):
                bw, bv, fi = wv_sb[f]
                col = 0
                for ch in cch:
                    h1 = ps1.tile([P, ch], f32, tag="h1", name=f"h1_{f}_{col}")
                    h2 = ps1.tile([P, ch], f32, tag="h2", name=f"h2_{f}_{col}")
                    for k in range(KT):
                        nc.tensor.matmul(h1[:], bw[:, fi, k, :],
                                         xt_sb[:, k, col:col + ch],
                                         start=(k == 0), stop=(k == KT - 1))
                    for k in range(KT):
                        nc.tensor.matmul(h2[:], bv[:, fi, k, :],
                                         xt_sb[:, k, col:col + ch],
                                         start=(k == 0), stop=(k == KT - 1))
                    g1 = gtmp.tile([P, ch], f32, tag="g1", name=f"g1_{f}_{col}")
                    nc.scalar.activation(g1[:], h1[:],
                                         mybir.ActivationFunctionType.Gelu)
                    nc.vector.tensor_mul(glu_sb[:, f, col:col + ch], g1[:], h2[:])
                    col += ch

            # ---- phase 2: outT[H, C] = W2.T @ gluT ----
            for h in range(HT):
                col = 0
                cch_h = cch
                if h >= HT - 2:
                    cch_h = []
                    for ch in cch:
                        if ch > 256:
                            cch_h += [ch - ch // 2, ch // 2]
                        else:
                            cch_h.append(ch)
                for ch in cch_h:
                    o_ps = ps2.tile([P, ch], f32, tag="ops", name=f"o_{h}_{col}")
                    for k in range(FT):
                        nc.tensor.matmul(o_ps[:],
                                         w2_sb[:, k, h * P:(h + 1) * P],
                                         glu_sb[:, k, col:col + ch],
                                         start=(k == 0), stop=(k == FT - 1))
                    o_sb = osb_pool.tile([P, ch], f32, tag="osb",
                                         name=f"os_{h}_{col}")
                    eng = nc.sync if (h + col // 256) % 2 == 0 else nc.scalar
                    nc.vector.tensor_copy(o_sb[:], o_ps[:])
                    if h == HT - 1:
                        # final h-tile: split each chunk across both HWDGE
                        # queues so the end-of-kernel DMA drain is halved
                        half = ch // 2
                        nc.sync.dma_start(
                            out=out_d.ap()[h * P:(h + 1) * P, col:col + half],
                            in_=o_sb[:, :half])
                        nc.scalar.dma_start(
                            out=out_d.ap()[h * P:(h + 1) * P,
                                           col + half:col + ch],
                            in_=o_sb[:, half:ch])
                    else:
                        eng.dma_start(
                            out=out_d.ap()[h * P:(h + 1) * P, col:col + ch],
                            in_=o_sb[:])
                    col += ch

    nc.compile()
    return nc


def kernel(x, top_weights, w1, v1, w2, top_experts):
    global LAST_EXEC_NS, LAST_MEAN_EXEC_NS, LAST_RESULTS

    from concourse.bass_utils import run_bass_kernel_spmd

    npdt = np.float16

    x = np.asarray(x)
    bsz, q_len, hidden = x.shape
    T = bsz * q_len
    x2 = np.ascontiguousarray(x.reshape(T, hidden).astype(np.float32, copy=False))
    te = np.asarray(top_experts).astype(np.int64, copy=False)
    tw = np.asarray(top_weights).astype(np.float32, copy=False)
    w1r = np.asarray(w1, dtype=np.float32).reshape(E, F, H)
    v1r = np.asarray(v1, dtype=np.float32).reshape(E, F, H)
    w2r = np.asarray(w2, dtype=np.float32).reshape(E, F, H)

    # Host-side dispatch: combine weights per (token, expert) summed over
    # top-k slots (handles duplicate experts within a token's top-k).
    cw = np.zeros((T, E), np.float32)
    rows = np.repeat(np.arange(T), TOPK)
    np.add.at(cw, (rows, te.reshape(-1)), tw.reshape(-1))

    ids = [np.nonzero((te == e).any(axis=1))[0] for e in range(E)]
    counts = [len(i) for i in ids]
    C = max(256, -(-max(counts) // P) * P)

    in_maps = []
    for e in range(E):
        xg = np.zeros((C, H), npdt)
        ce = counts[e]
        if ce:
            xg[:ce] = x2[ids[e]].astype(npdt)
        # xt[p, k, c] = xg[c, k*128+p]
        xt = np.ascontiguousarray(xg.reshape(C, KT, P).transpose(2, 1, 0))
        # wv[p, f, j, k, q] = Wj[e][f*128+q, k*128+p]
        w1t = w1r[e].astype(npdt).reshape(FT, P, KT, P).transpose(3, 0, 2, 1)
        v1t = v1r[e].astype(npdt).reshape(FT, P, KT, P).transpose(3, 0, 2, 1)
        wv = np.ascontiguousarray(np.stack([w1t, v1t], axis=2))
        # w2h[p, s, h] = W2[e][s*128+p, h]
        w2h = np.ascontiguousarray(
            w2r[e].astype(npdt).reshape(FT, P, H).transpose(1, 0, 2))
        in_maps.append({"xt": xt, "wv": wv, "w2": w2h})

    nc = _build_program(C)

    trace = os.environ.get("KERNEL_TRACE", "") == "1"
    if trace:
        _install_trace_shim()
        res = run_bass_kernel_spmd(nc, in_maps, list(range(E)),
                                   trace=True, trace_cores=list(range(E)))
        LAST_EXEC_NS = res.exec_time_ns
        LAST_MEAN_EXEC_NS = res.mean_exec_time_ns
        LAST_RESULTS = res
    else:
        res = run_bass_kernel_spmd(nc, in_maps, list(range(E)))

    # Host-side combine: scale each expert's rows by its routing weight and
    # scatter-add back to token order.
    out = np.zeros((T, H), np.float32)
    for e in range(E):
        ce = counts[e]
        if not ce:
            continue
        oe = res.results[e]["out"][:, :ce].T  # [ce, H]
        out[ids[e]] += oe * cw[ids[e], e][:, None]

    return out.reshape(bsz, q_len, hidden).astype(np.float32, copy=False)
